# revision 39
# baseline (speedup 1.0000x reference)
"""Fused QKV projection (dense transformer attention prologue) on 8 TRN2 NeuronCores.

Reference computation:
    qkv = hidden_states @ concat([Wq, Wk, Wv], axis=1) + concat([bq, bk, bv])
    q, k, v = split(qkv) -> each reshaped to [B, H, S, D]

Strategy: data-parallel over tokens (B*S = 16384 tokens -> 2048 per core).
Each core computes y^T[f, tok] = W^T x^T + b for its token slice:
  - W (fp32 in DRAM, replicated) is cast fp32->bf16 during the SWDGE DMA load.
  - x is transposed on-chip (PE identity transposes, or DMA-xbar in hybrid
    mode) and cast to bf16.
  - Matmuls run in bf16 with fp32 PSUM accumulation (K=1024 = 8 k-tiles).
  - The bias add is fused into the PSUM eviction (DVE tensor_scalar_add with a
    per-partition scalar), so it costs nothing extra.
  - Token-group-outer loop: xT for group g is only needed at phase g, so
    transposes for later groups overlap earlier phases' matmuls.
Host side only shards / concatenates / reassembles layouts.
"""

import numpy as np

import concourse.bass as bass
import concourse.mybir as mybir
from concourse import bacc
from concourse.bass import ds, ts
from concourse.bass_utils import run_bass_kernel_spmd
from concourse.masks import make_identity
from concourse.tile import TileContext

# Problem shapes (hardcoded per contract; kernel.py must be self-contained).
B, S = 4, 4096
HID = 1024
NH, HD = 16, 64
F = 3 * HID              # 3072 fused output features
NCORES = 8
TOK = B * S              # 16384
TOK_PC = TOK // NCORES   # 2048 tokens per core

P = 128
KT = HID // P            # 8 k tiles
XT = TOK_PC // P         # 16 x token tiles
NG = TOK_PC // 512       # 4 token groups of 512 (matmul N)
FT = F // P              # 24 f-tiles total
FH = 384                 # W column chunk (f per DMA)
NH_W = F // FH           # 8 W column chunks
FTH = FH // P            # 3 f-tiles per W chunk

FP32 = mybir.dt.float32
BF16 = mybir.dt.bfloat16


def _build_nc(repeat: int = 1, transpose_mode: str = "pe") -> bass.Bass:
    # Bacc (not raw Bass): its compile() runs move_matmul_waits_to_ldweights /
    # generate_event_semaphores, which walrus needs (1 sync-wait per inst).
    # `repeat` replays the main GEMM phase (benchmark-only work scaling).
    # transpose_mode:
    #   "pe"     - all x transposes on PE (fp32 in, DVE cast eviction)
    #   "hybrid" - x loaded bf16 (SWDGE cast DMA); token group 0 transposed
    #              on PE, groups 1-3 via DMA xbar during earlier phases
    nc = bacc.Bacc("TRN2")
    x = nc.declare_dram_parameter("x", [TOK_PC, HID], FP32, isOutput=False)
    w = nc.declare_dram_parameter("w", [HID, F], FP32, isOutput=False)
    bvec = nc.declare_dram_parameter("bvec", [F], FP32, isOutput=False)
    y = nc.declare_dram_parameter("y", [F, TOK_PC], FP32, isOutput=True)

    hybrid = transpose_mode == "hybrid"
    hybrid2 = transpose_mode == "hybrid2"
    hybrid3 = transpose_mode == "hybrid3"
    x_dt = BF16 if hybrid else FP32

    with TileContext(nc) as tc:
        with (
            tc.tile_pool(name="const", bufs=1) as const_pool,
            tc.tile_pool(name="xin", bufs=XT) as x_pool,
            tc.tile_pool(name="xtp", bufs=KT * NG) as xt_pool,
            tc.tile_pool(name="wsb", bufs=KT * NH_W) as w_pool,
            tc.tile_pool(name="ysb", bufs=8) as y_pool,
            tc.tile_pool(name="pstr", bufs=4, space="PSUM") as pstr_pool,
            tc.tile_pool(name="psmm", bufs=4, space="PSUM") as psmm_pool,
        ):
            # --- constants -------------------------------------------------
            ident = const_pool.tile([P, P], x_dt, name="ident")
            make_identity(nc, ident)
            identf = const_pool.tile([P, P], FP32, name="identf")
            make_identity(nc, identf)

            # bias laid out [partition, f_tile]: bias_sb[p, f] = bvec[f*128+p].
            # One contiguous [24, 128] DMA, then a PE transpose (K=24) into
            # PSUM and a DVE copy — lands in ~2us instead of 24 tiny DMAs.
            bias_rows = const_pool.tile([FT, P], FP32, name="bias_rows")
            nc.scalar.dma_start(
                out=bias_rows, in_=bvec.rearrange("(f p) -> f p", p=P)
            )
            bias_sb = const_pool.tile([P, FT], FP32, name="bias_sb")
            ps_b = pstr_pool.tile([P, 512], FP32, name="ps_bias", tag="pstr")
            nc.tensor.transpose(ps_b[:, :FT], bias_rows, identf[:FT, :FT])
            nc.vector.tensor_copy(bias_sb, ps_b[:, :FT])

            # --- input DMAs ------------------------------------------------
            # x token tiles [128, 1024]. "pe": fp32 alternating the two HWDGE
            # rings. "hybrid": bf16 inline-cast on the SWDGE ring.
            def _x_dma(t):
                xt = x_pool.tile([P, HID], x_dt, name=f"x{t}", tag="x")
                if hybrid:
                    eng = nc.gpsimd
                else:
                    eng = nc.sync if t % 2 == 0 else nc.scalar
                eng.dma_start(out=xt, in_=x[ts(t, P), :])
                return xt

            x_tiles = [_x_dma(t) for t in range(4)]

            # W tiles per (k, column-chunk): [128, 384] bf16, cast fp32->bf16
            # inline (SWDGE). First chunk (f 0:384, all 8 k) ships first so
            # f=0..2 matmuls can start early.
            w_half = {}

            def _w_dma(k, h):
                wt = w_pool.tile([P, FH], BF16, name=f"w{k}h{h}", tag="w")
                nc.gpsimd.dma_start(out=wt, in_=w[ts(k, P), ds(h * FH, FH)])
                w_half[(k, h)] = wt

            for k in range(KT):
                _w_dma(k, 0)

            if hybrid2:
                # DRAM->DRAM inline-cast staging for token groups 1..3
                # (group 0 is PE-transposed straight from the SBUF x tiles)
                for g in range(1, NG):
                    nc.gpsimd.dma_start(
                        out=x_bf_gd[g][:, :],
                        in_=x[ds(g * 512, 512), :],
                    )
            else:
                x_tiles += [_x_dma(t) for t in range(4, XT)]

            for h in range(1, NH_W):
                for k in range(KT):
                    _w_dma(k, h)

            # hybrid3: cast tiles t=4..15 to bf16 (DVE) and store them to the
            # DRAM staging buffer (ACT ring). Emitted lazily inside phase g0
            # (see _emit_stage below) so the casts interleave with evictions
            # on the DVE FIFO instead of blocking them.
            # DRAM staging tiles, one per token group 1..3. Pool tiles (not
            # raw dram_tensor) so Tile tracks the store->transpose RAW dep.
            xbf_pool = None
            x_bf_gd = {}
            if hybrid2 or hybrid3:
                xbfd_pool = tc.alloc_tile_pool(name="xbfd", bufs=3, space="DRAM")
                for g in range(1, NG):
                    x_bf_gd[g] = xbfd_pool.tile(
                        [512, HID], BF16, name=f"xbfd{g}", tag="xbfd"
                    )
            if hybrid3:
                xbf_pool = tc.alloc_tile_pool(name="xbf", bufs=4)

            def _emit_stage(t):
                xb = xbf_pool.tile([P, HID], BF16, name=f"xb{t}", tag="xb")
                nc.vector.tensor_copy(xb, x_tiles[t])
                g, i = divmod(t, 4)
                nc.scalar.dma_start(out=x_bf_gd[g][ts(i, P), :], in_=xb)

            # --- x transpose ----------------------------------------------
            # xT tile (k, g) holds x^T[k*128:(k+1)*128, g*512:(g+1)*512] bf16.
            xT = {}

            def _transpose_group_pe(g, x_major=False):
                # x_major: iterate source tiles outermost (half the k range
                # at a time so only 4 pstr banks are open) — the PE never
                # stalls waiting for the later x tiles of the group.
                ps_of, bf_of = {}, {}
                for k in range(KT):
                    bf_of[k] = xt_pool.tile(
                        [P, 512], BF16, name=f"xT{g}_{k}", tag="xT"
                    )
                if x_major:
                    for khalf in range(2):
                        ks = range(4 * khalf, 4 * khalf + 4)
                        for k in ks:
                            ps_of[k] = pstr_pool.tile(
                                [P, 512], x_dt, name=f"ps{g}_{k}", tag="pstr"
                            )
                        for i in range(4):
                            for k in ks:
                                nc.tensor.transpose(
                                    ps_of[k][:, ts(i, P)],
                                    x_tiles[4 * g + i][:, ts(k, P)],
                                    ident,
                                )
                        for k in ks:
                            nc.vector.tensor_copy(bf_of[k], ps_of[k])
                else:
                    for k in range(KT):
                        ps = pstr_pool.tile(
                            [P, 512], x_dt, name=f"ps{g}_{k}", tag="pstr"
                        )
                        for i in range(4):
                            nc.tensor.transpose(
                                ps[:, ts(i, P)],
                                x_tiles[4 * g + i][:, ts(k, P)],
                                ident,
                            )
                        nc.vector.tensor_copy(bf_of[k], ps)
                for k in range(KT):
                    xT[(k, g)] = bf_of[k]

            def _transpose_group_xbar(g):
                # bf16 SBUF->SBUF xbar transposes on the SP ring (idle after
                # the x loads); needed only by phase g, so there's huge slack.
                for k in range(KT):
                    xt_bf = xt_pool.tile([P, 512], BF16, name=f"xT{g}_{k}", tag="xT")
                    for i in range(4):
                        nc.sync.dma_start(
                            out=xt_bf[:, ts(i, P)],
                            in_=x_tiles[4 * g + i][:, ts(k, P)],
                            transpose=True,
                        )
                    xT[(k, g)] = xt_bf

            def _transpose_group_xbar_dram(g):
                # one [512, 128] DRAM->SBUF xbar transpose per k tile: 8 ops
                # per group instead of 32, reading the bf16 staging copy.
                for k in range(KT):
                    xt_bf = xt_pool.tile([P, 512], BF16, name=f"xT{g}_{k}", tag="xT")
                    nc.sync.dma_start(
                        out=xt_bf,
                        in_=x_bf_gd[g][:, ts(k, P)],
                        transpose=True,
                    )
                    xT[(k, g)] = xt_bf

            # Group 0 is transposed up front; groups 1..3 are emitted lazily
            # inside phase g0 ("pe"/"hybrid3") so the DVE FIFO interleaves
            # their PSUM evictions with the y evictions instead of blocking
            # them, and ("hybrid3") so the staging-store RAW dep is tracked.
            for g in range(NG):
                if hybrid and g > 0:
                    _transpose_group_xbar(g)
                elif hybrid2 and g > 0:
                    _transpose_group_xbar_dram(g)
                elif (hybrid3 or transpose_mode == "pe") and g > 0:
                    pass
                else:
                    _transpose_group_pe(g, x_major=True)

            # --- main GEMM + fused bias + store ----------------------------
            # token-group-outer: phase g sweeps all 24 f-tiles for one group
            # of 512 tokens. y chunks [128, 512] stream out as soon as the
            # fused-bias eviction lands.
            for rep in range(repeat):
                for g in range(NG):
                    for f in range(FT):
                        acc = psmm_pool.tile(
                            [P, 512], FP32, name=f"acc{g}_{f}", tag="acc"
                        )
                        for k in range(KT):
                            nc.tensor.matmul(
                                acc,
                                w_half[(k, f // FTH)][:, ts(f % FTH, P)],
                                xT[(k, g)],
                                start=(k == 0),
                                stop=(k == KT - 1),
                            )
                        ych = y_pool.tile([P, 512], FP32, name=f"y{g}_{f}", tag="y")
                        nc.vector.tensor_scalar_add(ych, acc, bias_sb[:, f : f + 1])
                        nc.scalar.dma_start(
                            out=y[ts(f, P), ds(g * 512, 512)], in_=ych
                        )
                        if hybrid3 and rep == 0 and g == 0 and f < 12:
                            _emit_stage(4 + f)
                            if f % 4 == 3:
                                # group f//4+1 fully staged -> emit its
                                # xbar transposes now (RAW dep is tracked)
                                _transpose_group_xbar_dram(f // 4 + 1)
                        if (
                            transpose_mode == "pe"
                            and rep == 0
                            and g == 0
                            and f in (3, 9, 15)
                        ):
                            _transpose_group_pe({3: 1, 9: 2, 15: 3}[f])
            if xbf_pool is not None:
                xbf_pool.release()
            if hybrid2 or hybrid3:
                xbfd_pool.release()

    nc.finalize()  # runs Bacc.compile(): reg alloc + sync-wait legalization
    return nc


_NC_CACHE = {}

# test-harness hooks: set TRACE=True before calling kernel() to profile the
# run; the full BassKernelResults lands in LAST_RESULTS either way.
TRACE = False
LAST_RESULTS = None

# kernel-variant knob (A/B'd via sim_profile/bench; best is default)
TRANSPOSE_MODE = "pe"


def _get_nc(repeat: int = 1) -> bass.Bass:
    key = (repeat, TRANSPOSE_MODE)
    if key not in _NC_CACHE:
        _NC_CACHE[key] = _build_nc(repeat, TRANSPOSE_MODE)
    return _NC_CACHE[key]


def kernel(hidden_states, Wq, bq, Wk, bk, Wv, bv):
    hidden_states = np.asarray(hidden_states, dtype=np.float32)
    w = np.concatenate(
        [np.asarray(Wq, np.float32), np.asarray(Wk, np.float32), np.asarray(Wv, np.float32)],
        axis=1,
    )
    bvec = np.concatenate(
        [np.asarray(bq, np.float32), np.asarray(bk, np.float32), np.asarray(bv, np.float32)]
    )

    x = np.ascontiguousarray(hidden_states.reshape(TOK, HID))
    in_maps = [
        {"x": x[c * TOK_PC : (c + 1) * TOK_PC], "w": w, "bvec": bvec}
        for c in range(NCORES)
    ]

    nc = _get_nc()
    res = run_bass_kernel_spmd(nc, in_maps, list(range(NCORES)), trace=TRACE)
    global LAST_RESULTS
    LAST_RESULTS = res
    outs = res.results

    q = np.empty((B, NH, S, HD), np.float32)
    k = np.empty((B, NH, S, HD), np.float32)
    v = np.empty((B, NH, S, HD), np.float32)
    for c in range(NCORES):
        yT = np.asarray(outs[c]["y"])             # [3072, 2048]
        part = yT.reshape(3, NH, HD, TOK_PC)      # [qkv, h, d, tok]
        b_i, s_i = divmod(c, S // TOK_PC)
        s0 = s_i * TOK_PC
        q[b_i, :, s0 : s0 + TOK_PC, :] = part[0].transpose(0, 2, 1)
        k[b_i, :, s0 : s0 + TOK_PC, :] = part[1].transpose(0, 2, 1)
        v[b_i, :, s0 : s0 + TOK_PC, :] = part[2].transpose(0, 2, 1)
    return q, k, v
